# revision 1
# baseline (speedup 1.0000x reference)
"""Trainium kernel for nn_AttLayer (GAT-style attention, 2 layers).

Strategy:
  - Data-parallel over batch (ego-graph) dim: B=256 -> 32 per core x 8 cores.
  - Layer 1 (4 heads, elu): computed in full per shard.
  - Layer 2: the reference returns out[:, 0, :] only, so the second
    attention layer collapses to a single-query attention at s=0:
      f1o = h1[:,0,:] @ (W1@a1), f2o = h1 @ (W1@a2)
      cw  = softmax(leaky_relu(f1o + f2o))
      out = (cw @ h1) @ W1 + b1
    This avoids materializing fts1=[B,S,128] and the [B,S,S] score matrix.
  - Heads and batch chunks are processed via lax.map to bound the live
    [chunk,S,S] score tensors (keeps HBM traffic + compile size down).
"""

import numpy as np

B, S, F = 256, 512, 256
HID, H0 = 128, 4
NCORES = 8
BSH = B // NCORES  # 32 batches per core


def _build_fn():
    import jax
    import jax.numpy as jnp
    from jax.sharding import Mesh, PartitionSpec as P
    from jax.experimental.shard_map import shard_map

    devs = jax.devices()[:NCORES]
    mesh = Mesh(np.array(devs), ("x",))

    CHUNK = 8  # batches processed per inner step; [8,512,512] scores = 8MB

    def shard_fn(xs, W0, a1_0, a2_0, b0, W1, a1_1, a2_1, b1):
        # xs: [BSH, S, F]; params replicated.
        u1 = W1 @ a1_1  # [H0*HID]
        u2 = W1 @ a2_1

        def one_head(carry, hparams):
            W, a1, a2, b = hparams

            def one_chunk(xc):
                fts = jnp.einsum("bsf,fo->bso", xc, W)  # [c,S,HID]
                f1 = jnp.einsum("bso,o->bs", fts, a1)
                f2 = jnp.einsum("bso,o->bs", fts, a2)
                logits = f1[:, :, None] + f2[:, None, :]  # [c,S,S]
                z = jax.nn.leaky_relu(logits, 0.2)
                coefs = jax.nn.softmax(z, axis=-1)
                vals = jnp.einsum("bst,bto->bso", coefs, fts) + b
                return jax.nn.elu(vals)  # [c,S,HID]

            xcs = xs.reshape(BSH // CHUNK, CHUNK, S, F)
            out = jax.lax.map(one_chunk, xcs)  # [nc,c,S,HID]
            return carry, out.reshape(BSH, S, HID)

        _, heads = jax.lax.scan(
            one_head, 0, (W0, a1_0, a2_0, b0)
        )  # [H0,BSH,S,HID]
        h1 = jnp.moveaxis(heads, 0, 2).reshape(BSH, S, H0 * HID)

        # Layer 2 at s=0 only.
        f1o = h1[:, 0, :] @ u1  # [BSH]
        f2o = jnp.einsum("bsc,c->bs", h1, u2)  # [BSH,S]
        lg = jax.nn.leaky_relu(f1o[:, None] + f2o, 0.2)
        cw = jax.nn.softmax(lg, axis=-1)  # [BSH,S]
        ctx = jnp.einsum("bs,bsc->bc", cw, h1)  # [BSH, H0*HID]
        return ctx @ W1 + b1  # [BSH, OUT]

    rep = P()
    fn = shard_map(
        shard_fn,
        mesh=mesh,
        in_specs=(P("x"), rep, rep, rep, rep, rep, rep, rep, rep),
        out_specs=P("x"),
        check_rep=False,
    )
    jfn = jax.jit(fn)
    return jax, mesh, jfn


_CACHE = {}


def kernel(x, W0, a1_0, a2_0, b0, W1, a1_1, a2_1, b1):
    if "fn" not in _CACHE:
        _CACHE["fn"] = _build_fn()
    jax, mesh, jfn = _CACHE["fn"]

    args = [
        np.asarray(a, np.float32)
        for a in (x, W0, a1_0, a2_0, b0, W1, a1_1, a2_1, b1)
    ]
    out = jfn(*args)
    out = np.asarray(jax.device_get(out), np.float32)
    return out


if __name__ == "__main__":
    rng = np.random.default_rng(0)
    ins = {
        "x": rng.standard_normal((B, S, F), dtype=np.float32),
        "W0": rng.standard_normal((H0, F, HID), dtype=np.float32) * 0.05,
        "a1_0": rng.standard_normal((H0, HID), dtype=np.float32) * 0.05,
        "a2_0": rng.standard_normal((H0, HID), dtype=np.float32) * 0.05,
        "b0": np.zeros((H0, HID), np.float32),
        "W1": rng.standard_normal((H0 * HID, 128), dtype=np.float32) * 0.05,
        "a1_1": rng.standard_normal((128,), dtype=np.float32) * 0.05,
        "a2_1": rng.standard_normal((128,), dtype=np.float32) * 0.05,
        "b1": np.zeros((128,), np.float32),
    }
    print(kernel(**ins).shape)
